# revision 2
# baseline (speedup 1.0000x reference)
"""Trainium2 Bass kernel for nn_Attention_88510686036327.

GQA attention (S=2048, DIM=4096, H=32 q-heads, KVH=8 kv-heads, D=128) with
RoPE and causal softmax, tensor-parallel across 8 NeuronCores: each core owns
1 kv-head + 4 q-heads (wq/wk/wv sharded on head dim, wo on input dim), and the
partial wo outputs are summed on the host.

Numerics: single-pass fp16 matmuls throughout (projections, QK^T, PV, wo) —
CPU simulation of this exact pipeline vs the fp32 reference gives rel-L2
~8e-3, comfortably under the 2e-2 gate. The only extra-precision spot is the
RoPE pair-swap matmul, computed from a bf16 hi/lo split of the fp32
projection (a single bf16 swap pass measures 2.7e-2 — over the gate).
RoPE and softmax run fp32 on DVE/ACT; weights are pre-scaled by 64 on host
(descale folded into existing eviction ops); softmax normalization via a
per-partition multiply of the exp tile on DVE.

Layouts (host-prepped): x, wq, wk, wv transposed so the contraction dim lands
on SBUF partitions with no on-device transposes; q/k/v produced directly in
the layouts the attention matmuls need.
"""
import sys

sys.path.insert(0, "/opt/trn_rl_repo")

import numpy as np

S = 2048
DIM = 4096
H = 32
KVH = 8
D = 128
N_CORES = 8
HPC = H // N_CORES          # q heads per core
MQ = HPC * D                # per-core q rows (512)
KT = DIM // 128             # contraction tiles (32)
SC = S // 512               # s-chunks (4)
WSCALE = 64.0
SQRT_D = float(np.sqrt(D))
NEG = -1e30

_CACHE = {}
LAST_RESULT = None


def _build(repeat=1):
    import concourse.bacc as bacc
    import concourse.mybir as mybir
    import concourse.tile as tile

    dt = mybir.dt
    f16, f32, bf16 = dt.float16, dt.float32, dt.bfloat16
    AX = mybir.AxisListType.X
    SUB = mybir.AluOpType.subtract
    ADD = mybir.AluOpType.add
    EXP = mybir.ActivationFunctionType.Exp

    nc = bacc.Bacc("TRN2", target_bir_lowering=False, debug=False)

    def din(name, shape, d=f16):
        return nc.dram_tensor(name, shape, d, kind="ExternalInput").ap()

    xp_d = din("xpack", [DIM, S])
    wqh_d = din("wqh", [DIM, MQ])
    wkh_d = din("wkh", [DIM, D])
    wv_d = din("wv", [DIM, D])
    wo_d = din("wot", [MQ, DIM])
    cos_d = din("cosf", [D, S], f32)
    sin_d = din("sinf", [D, S], f32)
    mask_d = din("masks", [4, 128, 512], f32)
    id_d = din("ident", [128, 128], f16)
    rm_d = din("rmat", [128, 128], bf16)
    y_d = nc.dram_tensor("y", [S, DIM], f16, kind="ExternalOutput").ap()

    xp_r = xp_d.rearrange("(kt p) s -> p kt s", p=128)
    wqh_r = wqh_d.rearrange("(kt p) m -> p kt m", p=128)
    wkh_r = wkh_d.rearrange("(kt p) m -> p kt m", p=128)
    wv_r = wv_d.rearrange("(kt p) m -> p kt m", p=128)
    wo_r = wo_d.rearrange("(kd p) n -> p kd n", p=128)

    with tile.TileContext(nc) as tc:
        with tc.tile_pool(name="persist", bufs=1) as pp:
            ident = pp.tile([128, 128], f16, name="ident")
            nc.sync.dma_start(ident[:], id_d)
            rmat = pp.tile([128, 128], bf16, name="rmat")
            nc.sync.dma_start(rmat[:], rm_d)
            maskt = []
            for j in range(4):
                mj = pp.tile([128, 512], f32, name=f"mask{j}", tag=f"mask{j}")
                maskt.append(mj)
            qh_s = pp.tile([128, HPC, S], f16, name="qh_s")
            kh_s = pp.tile([128, S], f16, name="kh_s")
            v_s = pp.tile([128, 16, 128], f16, name="v_s")

            for _rep in range(repeat):
                # ---------------- phase 1: projections + rope ----------------
                with (
                    tc.tile_pool(name="p1w", bufs=1) as p1w,
                    tc.tile_pool(name="p1x", bufs=3) as p1x,
                    tc.tile_pool(name="p1r", bufs=2) as p1r,
                    tc.tile_pool(name="ps1", bufs=1, space="PSUM") as ps1,
                ):
                    wqh = p1w.tile([128, KT, MQ], f16, name="wqh")
                    wkh = p1w.tile([128, KT, D], f16, name="wkh")
                    wv = p1w.tile([128, KT, D], f16, name="wv")
                    cosf = p1w.tile([128, S], f32, name="cosf")
                    sinf = p1w.tile([128, S], f32, name="sinf")

                    def rope_unit(psum, outh, ss):
                        """psum [128,512] raw proj -> rope'd fp16 slice."""
                        sb = p1r.tile([128, 512], f32, name="ropesb", tag="ropesb")
                        nc.scalar.mul(sb[:], psum[:], 1.0 / WSCALE)
                        sbh = p1r.tile([128, 512], bf16, name="ropesbh", tag="ropesbh")
                        nc.vector.tensor_copy(sbh[:], sb[:])
                        sbl = p1r.tile([128, 512], bf16, name="ropesbl", tag="ropesbl")
                        nc.vector.tensor_tensor(sbl[:], sb[:], sbh[:], SUB)
                        sw = ps1.tile([128, 512], f32, name="ropesw", tag="ropesw")
                        nc.tensor.matmul(sw[:], rmat[:], sbh[:], start=True, stop=False)
                        nc.tensor.matmul(sw[:], rmat[:], sbl[:], start=False, stop=True)
                        t1 = p1r.tile([128, 512], f32, name="ropet1", tag="ropet1")
                        nc.vector.tensor_mul(t1[:], sb[:], cosf[:, ss])
                        t2 = p1r.tile([128, 512], f32, name="ropet2", tag="ropet2")
                        nc.vector.tensor_mul(t2[:], sw[:], sinf[:, ss])
                        nc.vector.tensor_add(outh, t1[:], t2[:])  # fp16 out

                    for sc in range(SC):
                        ss = slice(sc * 512, (sc + 1) * 512)
                        qps = [
                            ps1.tile([128, 512], f32, name=f"qps{m}", tag=f"qps{m}")
                            for m in range(HPC)
                        ]
                        kps = ps1.tile([128, 512], f32, name="kps", tag="kps")
                        vps = ps1.tile([128, 512], f32, name="vps", tag="vps")
                        for kt in range(KT):
                            first, last = kt == 0, kt == KT - 1
                            if sc == 0:
                                nc.sync.dma_start(wqh[:, kt, :], wqh_r[:, kt, :])
                                nc.sync.dma_start(wkh[:, kt, :], wkh_r[:, kt, :])
                                nc.sync.dma_start(wv[:, kt, :], wv_r[:, kt, :])
                                if kt == 8:
                                    nc.sync.dma_start(cosf[:], cos_d)
                                    nc.sync.dma_start(sinf[:], sin_d)

                            xpt = p1x.tile([128, 512], f16, name="xpt", tag="xpt")
                            nc.sync.dma_start(xpt[:], xp_r[:, kt, ss])
                            for m in range(HPC):
                                wh = wqh[:, kt, m * 128 : (m + 1) * 128]
                                nc.tensor.matmul(qps[m][:], wh, xpt[:], start=first, stop=last)
                            nc.tensor.matmul(kps[:], wkh[:, kt, :], xpt[:], start=first, stop=last)
                            nc.tensor.matmul(vps[:], wv[:, kt, :], xpt[:], start=first, stop=last)
                        for m in range(HPC):
                            rope_unit(qps[m], qh_s[:, m, ss], ss)
                        rope_unit(kps, kh_s[:, ss], ss)
                        # V: evict fp16 then transpose to natural [s, d] layout
                        vsb = p1r.tile([128, 512], f16, name="vsb", tag="vsb")
                        nc.scalar.mul(vsb[:], vps[:], 1.0 / WSCALE)
                        vtp = ps1.tile([128, 512], f16, name="vtp", tag="vtp")
                        for j in range(4):
                            nc.tensor.transpose(
                                vtp[:, j * 128 : (j + 1) * 128],
                                vsb[:, j * 128 : (j + 1) * 128],
                                ident[:],
                            )
                        nc.vector.tensor_copy(v_s[:, sc * 4 : (sc + 1) * 4, :], vtp[:])

                # ---------------- phase 2: attention + wo ----------------
                # Software-pipelined over the 16 (qc, h) units: softmax(u)
                # is emitted before transpose+PV(u-1) so the PE fills the
                # exp-latency of unit u-1 with unit u's score matmuls.
                with (
                    tc.tile_pool(name="p2w", bufs=1) as p2w,
                    tc.tile_pool(name="p2a", bufs=2) as p2a,
                    tc.tile_pool(name="p2x", bufs=3) as p2x,
                    tc.tile_pool(name="ps2", bufs=2, space="PSUM") as ps2,
                ):
                    for j in range(4):
                        nc.sync.dma_start(maskt[j][:], mask_d[j])
                    wot = p2w.tile([128, HPC, DIM], f16, name="wot")
                    nc.sync.dma_start(wot[:], wo_r)

                    def emit_softmax(qc, h):
                        nkc = qc + 1
                        exps = []
                        for qtl in range(4):
                            qt = qc * 4 + qtl
                            qsl = slice(qt * 128, (qt + 1) * 128)
                            scsb = p2a.tile([128, S], f32, name="scsb", tag="scsb")
                            qh_sl = qh_s[:, h, qsl]
                            kext = qt * 128 + 128
                            for kc in range(nkc):
                                w = min(512, kext - kc * 512)
                                ks = slice(kc * 512, kc * 512 + w)
                                sps = ps2.tile([128, 512], f32, name="sps", tag="sps", bufs=3)
                                nc.tensor.matmul(sps[:, :w], qh_sl, kh_s[:, ks], start=True, stop=True)
                                if kc == qc:
                                    nc.vector.tensor_tensor(scsb[:, ks], sps[:, :w], maskt[qtl][:, :w], ADD)
                                else:
                                    nc.scalar.copy(scsb[:, ks], sps[:, :w])
                            mrow = p2a.tile([128, 1], f32, name="mrow", tag="mrow")
                            nc.vector.reduce_max(mrow[:], scsb[:, 0:kext], axis=AX)
                            bias = p2a.tile([128, 1], f32, name="bias", tag="bias")
                            nc.vector.tensor_scalar_mul(bias[:], mrow[:], -SQRT_D)
                            expt = p2a.tile([128, S], f16, name=f"exp{qtl}", tag=f"exp{qtl}", bufs=5)
                            tot = p2a.tile([128, 1], f32, name="tot", tag="tot")
                            nc.scalar.activation(
                                expt[:, 0:kext],
                                scsb[:, 0:kext],
                                EXP,
                                bias=bias[:],
                                scale=SQRT_D,
                                accum_out=tot[:],
                            )
                            rr = p2a.tile([128, 1], f32, name="rr", tag="rr")
                            nc.vector.reciprocal(rr[:], tot[:])
                            nc.vector.tensor_scalar_mul(
                                expt[:, 0:kext], expt[:, 0:kext], rr[:]
                            )
                            exps.append(expt)
                        return exps

                    def emit_transpv(qc, h, exps):
                        # P transpose + PV; diag chunks skip all-zero quarters
                        ops_ = ps2.tile([128, 512], f32, name="ops", tag="ops")
                        nkt = 4 * (qc + 1)
                        pend = [None] * nkt  # (pt, q0)
                        for kt in range(nkt):
                            j0 = max(0, kt - 4 * qc)
                            q0 = j0 * 128
                            tps = ps2.tile([128, 512], f16, name="tps", tag="tps")
                            for qtl in range(j0, 4):
                                nc.tensor.transpose(
                                    tps[:, qtl * 128 : (qtl + 1) * 128],
                                    exps[qtl][:, kt * 128 : (kt + 1) * 128],
                                    ident[:],
                                )
                            pt = p2x.tile([128, 512], f16, name="pt", tag="pt")
                            if kt % 2 == 0:
                                nc.vector.tensor_copy(pt[:, q0:], tps[:, q0:])
                            else:
                                nc.scalar.copy(pt[:, q0:], tps[:, q0:])
                            pend[kt] = (pt, q0)
                            if kt > 0:
                                ppt, pq0 = pend[kt - 1]
                                nc.tensor.matmul(
                                    ops_[:, pq0:], v_s[:, kt - 1, :], ppt[:, pq0:],
                                    start=(kt == 1), stop=False,
                                )
                        ppt, pq0 = pend[nkt - 1]
                        nc.tensor.matmul(
                            ops_[:, pq0:], v_s[:, nkt - 1, :], ppt[:, pq0:],
                            start=(nkt == 1), stop=True,
                        )
                        at = p2a.tile([128, 512], f16, name=f"attn{h}", tag=f"attn{h}")
                        nc.vector.tensor_copy(at[:], ops_[:])
                        return at

                    def emit_wo(qc, attn_t):
                        for stl in range(4):
                            sl = slice(stl * 128, (stl + 1) * 128)
                            row0 = qc * 512 + stl * 128
                            for dc in range(8):
                                yps = ps2.tile([128, 512], f32, name="yps", tag="yps", bufs=1)
                                for h in range(HPC):
                                    nc.tensor.matmul(
                                        yps[:],
                                        attn_t[h][:, sl],
                                        wot[:, h, dc * 512 : (dc + 1) * 512],
                                        start=(h == 0),
                                        stop=(h == HPC - 1),
                                    )
                                ysb = p2x.tile([128, 512], f16, name="ysb", tag="ysb")
                                if dc % 2 == 0:
                                    nc.vector.tensor_scalar_mul(ysb[:], yps[:], 1.0 / WSCALE)
                                else:
                                    nc.scalar.mul(ysb[:], yps[:], 1.0 / WSCALE)
                                nc.sync.dma_start(
                                    y_d[row0 : row0 + 128, dc * 512 : (dc + 1) * 512], ysb[:]
                                )

                    units = [(qc, h) for qc in range(SC) for h in range(HPC)]
                    attn_map = {qc: [None] * HPC for qc in range(SC)}
                    DEPTH = 4
                    pend = []  # [(unit, exps), ...] awaiting transpv
                    def retire(u, exps_u):
                        attn_map[u[0]][u[1]] = emit_transpv(*u, exps_u)
                        if u[1] == HPC - 1:
                            emit_wo(u[0], attn_map[u[0]])
                    for u in units:
                        exps_u = emit_softmax(*u)
                        pend.append((u, exps_u))
                        if len(pend) > DEPTH:
                            retire(*pend.pop(0))
                    for item in pend:
                        retire(*item)
    nc.compile()
    return nc


def _get_nc(repeat=1):
    key = ("nc", repeat)
    if key not in _CACHE:
        _CACHE[key] = _build(repeat)
    return _CACHE[key]


def _prep_inputs(x, wq, wk, wv, wo, freqs_cis):
    """Host-side shard + layout prep. Returns in_maps (one dict per core)."""
    f16 = np.float16
    f32 = np.float32
    x = np.asarray(x, f32)
    wq = np.asarray(wq, f32)
    wk = np.asarray(wk, f32)
    wv = np.asarray(wv, f32)
    wo = np.asarray(wo, f32)
    fc = np.asarray(freqs_cis, f32)

    xpack = np.ascontiguousarray(x.T).astype(f16)       # [DIM, S]

    # rope tables in [d, s] layout; sin carries the pair-swap signs
    cosf = np.empty((D, S), f32)
    sinf = np.empty((D, S), f32)
    c = fc[:, :, 0].T                                   # [64, S]
    s = fc[:, :, 1].T
    cosf[0::2] = c
    cosf[1::2] = c
    sinf[0::2] = -s
    sinf[1::2] = s

    masks = np.empty((4, 128, 512), f32)
    q_i = np.arange(128)[:, None]
    k_i = np.arange(512)[None, :]
    for j in range(4):
        masks[j] = np.where(k_i <= 128 * j + q_i, 0.0, NEG)

    ident = np.eye(128, dtype=f16)
    import ml_dtypes
    rmat = np.zeros((128, 128), ml_dtypes.bfloat16)
    ii = np.arange(0, 128, 2)
    rmat[ii + 1, ii] = 1.0   # lhsT[2i+1, 2i]=1 -> out[2i] = in[2i+1]
    rmat[ii, ii + 1] = 1.0   # lhsT[2i, 2i+1]=1 -> out[2i+1] = in[2i]

    in_maps = []
    for cidx in range(N_CORES):
        hs = slice(cidx * MQ, (cidx + 1) * MQ)
        ks = slice(cidx * D, (cidx + 1) * D)
        wqh = np.ascontiguousarray((wq[hs] * WSCALE).T).astype(f16)  # [DIM, 512]
        wkh = np.ascontiguousarray((wk[ks] * WSCALE).T).astype(f16)  # [DIM, 128]
        wvT = np.ascontiguousarray((wv[ks] * WSCALE).T).astype(f16)
        woT = np.ascontiguousarray((wo[:, hs] * WSCALE).T).astype(f16)  # [512, DIM]
        in_maps.append(
            {
                "xpack": xpack,
                "wqh": wqh,
                "wkh": wkh,
                "wv": wvT, "wot": woT,
                "cosf": cosf, "sinf": sinf,
                "masks": masks, "ident": ident, "rmat": rmat,
            }
        )
    return in_maps


def kernel(**inputs):
    global LAST_RESULT
    from concourse.bass_utils import run_bass_kernel_spmd

    in_maps = _prep_inputs(
        inputs["x"], inputs["wq"], inputs["wk"], inputs["wv"], inputs["wo"],
        inputs["freqs_cis"],
    )
    nc = _get_nc()
    r = run_bass_kernel_spmd(nc, in_maps, core_ids=list(range(N_CORES)))
    LAST_RESULT = r
    y = np.zeros((S, DIM), np.float32)
    for cidx in range(N_CORES):
        y += r.results[cidx]["y"].astype(np.float32)
    return y


if __name__ == "__main__":
    rng = np.random.default_rng(0)
    demo = {
        "x": rng.standard_normal((S, DIM)).astype(np.float32),
        "wq": (rng.standard_normal((H * D, DIM)) * 0.02).astype(np.float32),
        "wk": (rng.standard_normal((KVH * D, DIM)) * 0.02).astype(np.float32),
        "wv": (rng.standard_normal((KVH * D, DIM)) * 0.02).astype(np.float32),
        "wo": (rng.standard_normal((DIM, H * D)) * 0.02).astype(np.float32),
        "freqs_cis": np.stack(
            [
                np.cos(np.outer(np.arange(S), 1.0 / 10000.0 ** (np.arange(0, D, 2) / D))),
                np.sin(np.outer(np.arange(S), 1.0 / 10000.0 ** (np.arange(0, D, 2) / D))),
            ],
            axis=-1,
        ).astype(np.float32),
    }
    y = kernel(**demo)
    print("ok", y.shape, y.dtype)
